# revision 5
# baseline (speedup 1.0000x reference)
"""AVEQA dense transformer on 8 Trainium2 NeuronCores.

Sharding: data-parallel over batch (16 items -> 2 per core). Each core runs
both 12-layer encoders (pre-norm "contextual" + post-norm BERT) on its 2
sequences, computes the span heads on-device (argmax + dynamic-slice gather),
and its slice of the DMLM projection (16 rows x 30522) with on-device softmax.

Numerics: fp16 matmul operands, fp32 PSUM accumulate, fp32 residual stream,
fp32 layernorm/softmax stats. Pre-norm LN gain/bias folded into the following
weights on the host; head biases (constant shifts) added on the host.
"""
import numpy as np
from contextlib import ExitStack

B, S, H, L, NH, V = 16, 128, 768, 12, 12, 30522
DK = H // NH
FF = 4 * H
SPAN = 4
NCORES = 8
BP = B // NCORES          # batch items per core (2)
KT = H // 128             # 6 k-tiles over H
FKT = FF // 128           # 24 k-tiles over FF
VTILE = 512
NVT = (V + VTILE - 1) // VTILE   # 60 projection column tiles (last = 314)

L_CTX = L
L_BERT = L

_BUILT = {}


def _build(l_ctx, l_bert):
    import concourse.bass as bass
    import concourse.tile as tile
    from concourse import bacc, mybir
    from concourse.masks import make_identity

    f32 = mybir.dt.float32
    f16 = mybir.dt.float16
    u32 = mybir.dt.uint32
    AF = mybir.ActivationFunctionType
    OP = mybir.AluOpType

    nc = bacc.Bacc("TRN2", target_bir_lowering=False, debug=False)

    def dram(name, shape, dt=f16, kind="ExternalInput"):
        return nc.dram_tensor(name, list(shape), dt, kind=kind).ap()

    d = {}
    d['x0'] = dram('x0', (BP, S, H), f32)
    d['y0'] = dram('y0', (BP, S, H), f32)
    for p in ('c', 'b'):
        ll = l_ctx if p == 'c' else l_bert
        d[p + 'qkw'] = dram(p + 'qkw', (ll, H, 2 * H))          # q,k columns
        d[p + 'qkb'] = dram(p + 'qkb', (ll, 128, 12), f32)      # per-tile bias cols
        d[p + 'vw'] = dram(p + 'vw', (ll, H, H))
        d[p + 'ow'] = dram(p + 'ow', (ll, H, H))
        d[p + 'ob'] = dram(p + 'ob', (ll, H))
        d[p + 'f1w'] = dram(p + 'f1w', (ll, H, FF))
        d[p + 'f1b'] = dram(p + 'f1b', (ll, 128, FKT), f32)
        d[p + 'f2w'] = dram(p + 'f2w', (ll, FF, H))
        d[p + 'f2b'] = dram(p + 'f2b', (ll, H))
    d['bln'] = dram('bln', (l_bert, 4, H), f32)                 # g1,b1,g2,b2
    d['wb'] = dram('wb', (H, 1))
    d['we1'] = dram('we1', (H, 1))
    d['we2'] = dram('we2', (H, 1))
    d['cls'] = dram('cls', (H, 2))
    d['sel'] = dram('sel', (BP, S, SPAN))
    d['pw'] = dram('pw', (H, V))
    d['pb'] = dram('pb', (1, V))
    d['o_begin'] = dram('o_begin', (BP, S), f32, kind="ExternalOutput")
    d['o_end'] = dram('o_end', (BP, 32), f32, kind="ExternalOutput")
    d['o_noans'] = dram('o_noans', (BP, 2), f32, kind="ExternalOutput")
    d['o_bert'] = dram('o_bert', (BP * SPAN, V), f16, kind="ExternalOutput")
    d['o_ctx'] = dram('o_ctx', (BP * SPAN, V), f16, kind="ExternalOutput")

    def bcast_ap(v, p=128):
        # replicate a [N] dram row across p partitions
        return bass.AP(tensor=v.tensor, offset=v.offset, ap=[[0, p]] + list(v.ap))

    with ExitStack() as ctx:
        tc = ctx.enter_context(tile.TileContext(nc))
        const = ctx.enter_context(tc.tile_pool(name="const", bufs=1))
        persist = ctx.enter_context(tc.tile_pool(name="persist", bufs=1))
        res = ctx.enter_context(tc.tile_pool(name="res", bufs=2))
        wp = ctx.enter_context(tc.tile_pool(name="wp", bufs=2))
        actT = ctx.enter_context(tc.tile_pool(name="actT", bufs=1))
        actS = ctx.enter_context(tc.tile_pool(name="actS", bufs=2))
        sm = ctx.enter_context(tc.tile_pool(name="sm", bufs=4))
        pm_mm = ctx.enter_context(tc.tile_pool(name="pm_mm", bufs=2, space="PSUM"))
        pm_at = ctx.enter_context(tc.tile_pool(name="pm_at", bufs=2, space="PSUM"))
        pm_tp = ctx.enter_context(tc.tile_pool(name="pm_tp", bufs=2, space="PSUM"))
        pm_sm = ctx.enter_context(tc.tile_pool(name="pm_sm", bufs=1, space="PSUM"))

        ident = const.tile([128, 128], f16, tag="ident")
        make_identity(nc, ident)
        ones_col = const.tile([128, 1], f16, tag="ones_col")
        nc.vector.memset(ones_col, 1.0)
        ones_row = const.tile([1, 128], f16, tag="ones_row")
        nc.vector.memset(ones_row, 1.0)
        ones_16 = const.tile([1, 16], f16, tag="ones_16")
        nc.vector.memset(ones_16, 1.0)
        eps1 = const.tile([128, 1], f32, tag="eps1")
        nc.vector.memset(eps1, 1e-12)

        def ln_stats(x):
            st = sm.tile([128, 3, 6], f32, tag="st")
            for i in range(3):
                nc.vector.bn_stats(st[:, i], x[:, i * 256:(i + 1) * 256])
            mv = sm.tile([128, 2], f32, tag="mv")
            nc.vector.bn_aggr(mv, st)
            lnv = sm.tile([128, 1], f32, tag="lnv")
            nc.scalar.activation(lnv, mv[:, 1:2], AF.Ln, bias=eps1)
            rstd = sm.tile([128, 1], f32, tag="rstd")
            nc.scalar.activation(rstd, lnv, AF.Exp, scale=-0.5)
            return mv[:, 0:1], rstd

        def transpose_in(x16, dstT, s):
            # x16 [128,768] f16 token-major -> dstT[kt][:, s*128:(s+1)*128]
            for kt in range(KT):
                tp = pm_tp.tile([128, 128], f16, tag="tp")
                nc.tensor.transpose(tp, x16[:, kt * 128:(kt + 1) * 128], ident)
                nc.vector.tensor_copy(dstT[kt][:, s * 128:(s + 1) * 128], tp)

        def encoder_layer(pfx, l, xs, xTs, prenorm):
            """One transformer layer for both sequences.
            xs: per-seq residual [128,768] f32 tiles.
            xTs: for post-norm, the f16 feature-major current activations.
            Returns (new_xs, new_y16s, new_xTs)."""
            W = lambda n: d[pfx + n]
            qkw = [wp.tile([128, 2 * H], f16, tag=f"qkw{kt}") for kt in range(KT)]
            vw = [wp.tile([128, H], f16, tag=f"vw{kt}") for kt in range(KT)]
            ow = [wp.tile([128, H], f16, tag=f"ow{kt}") for kt in range(KT)]
            f1w = [wp.tile([128, FF], f16, tag=f"f1w{kt}") for kt in range(KT)]
            f2w = [wp.tile([128, H], f16, tag=f"f2w{kt}") for kt in range(FKT)]
            for kt in range(KT):
                nc.sync.dma_start(qkw[kt], W('qkw')[l, kt * 128:(kt + 1) * 128])
                nc.sync.dma_start(vw[kt], W('vw')[l, kt * 128:(kt + 1) * 128])
                nc.sync.dma_start(ow[kt], W('ow')[l, kt * 128:(kt + 1) * 128])
                nc.sync.dma_start(f1w[kt], W('f1w')[l, kt * 128:(kt + 1) * 128])
            for kt in range(FKT):
                nc.sync.dma_start(f2w[kt], W('f2w')[l, kt * 128:(kt + 1) * 128])
            qkb = wp.tile([128, 12], f32, tag="qkb")
            nc.sync.dma_start(qkb, W('qkb')[l])
            f1b = wp.tile([128, FKT], f32, tag="f1b")
            nc.sync.dma_start(f1b, W('f1b')[l])
            obr = wp.tile([1, H], f16, tag="obr")
            nc.sync.dma_start(obr, W('ob')[l][None, :])
            f2br = wp.tile([1, H], f16, tag="f2br")
            nc.sync.dma_start(f2br, W('f2b')[l][None, :])
            if not prenorm:
                grep = [wp.tile([128, H], f32, tag=f"grep{i}") for i in range(4)]
                for i in range(4):
                    nc.sync.dma_start(grep[i], bcast_ap(d['bln'][l, i]))

            if prenorm:
                # x_hat = (x-mu)*rstd; g/b folded into qkw/qkb on host
                xT = [actT.tile([128, 2 * 128], f16, tag=f"xT{kt}") for kt in range(KT)]
                for s in range(BP):
                    mu, rstd = ln_stats(xs[s])
                    xh = actS.tile([128, H], f16, tag="xh")
                    nc.vector.tensor_scalar(xh, xs[s], mu, rstd, OP.subtract, OP.mult)
                    transpose_in(xh, xT, s)
            else:
                xT = xTs

            # QKV (q,k feature-major; both seqs batched N=256)
            qk = [actT.tile([128, 2 * 128], f16, tag=f"qk{nt}") for nt in range(12)]
            for nt in range(12):
                ps = pm_mm.tile([128, 256], f32, tag="mm")
                for kt in range(KT):
                    nc.tensor.matmul(ps, qkw[kt][:, nt * 128:(nt + 1) * 128], xT[kt],
                                     start=(kt == 0), stop=(kt == KT - 1))
                nc.vector.tensor_scalar(qk[nt], ps, qkb[:, nt:nt + 1], None, OP.add)
            # V (token-major per seq)
            vsb = []
            for s in range(BP):
                vt = actT.tile([128, H], f16, tag=f"v{s}")
                for half in range(2):
                    ps = pm_mm.tile([128, 384], f32, tag="mm")
                    for kt in range(KT):
                        nc.tensor.matmul(ps, xT[kt][:, s * 128:(s + 1) * 128],
                                         vw[kt][:, half * 384:(half + 1) * 384],
                                         start=(kt == 0), stop=(kt == KT - 1))
                    nc.vector.tensor_copy(vt[:, half * 384:(half + 1) * 384], ps)
                vsb.append(vt)

            # attention per seq; attn_out token-major, then transposed
            aoT = [actT.tile([128, 2 * 128], f16, tag=f"aoT{kt}") for kt in range(KT)]
            for s in range(BP):
                ao = actS.tile([128, H], f16, tag="ao")
                for t in range(6):
                    qts = qk[t][:, s * 128:(s + 1) * 128]
                    kts = qk[6 + t][:, s * 128:(s + 1) * 128]
                    psS = pm_at.tile([128, 256], f32, tag="at")
                    nc.tensor.matmul(psS[:, 0:128], kts[0:64, :], qts[0:64, :],
                                     start=True, stop=True)
                    nc.tensor.matmul(psS[:, 128:256], kts[64:128, :], qts[64:128, :],
                                     start=False, stop=True, tile_position=(64, 0),
                                     skip_group_check=True)
                    expP = actS.tile([128, 256], f16, tag="expP")
                    nc.scalar.activation(expP, psS, AF.Exp, scale=0.125)
                    psN = pm_sm.tile([128, 2], f32, tag="psN")
                    nc.tensor.matmul(psN[:, 0:1], expP[:, 0:128], ones_col,
                                     start=True, stop=True)
                    nc.tensor.matmul(psN[:, 1:2], expP[:, 128:256], ones_col,
                                     start=False, stop=True, skip_group_check=True)
                    rc = sm.tile([128, 2], f32, tag="rc")
                    nc.vector.reciprocal(rc, psN)
                    psV = pm_at.tile([128, 128], f32, tag="at")
                    nc.tensor.matmul(psV[:, 0:64], expP[:, 0:128],
                                     vsb[s][:, (2 * t) * 64:(2 * t + 1) * 64],
                                     start=True, stop=True)
                    nc.tensor.matmul(psV[:, 64:128], expP[:, 128:256],
                                     vsb[s][:, (2 * t + 1) * 64:(2 * t + 2) * 64],
                                     start=False, stop=True, skip_group_check=True)
                    nc.vector.tensor_scalar(ao[:, (2 * t) * 64:(2 * t + 1) * 64],
                                            psV[:, 0:64], rc[:, 0:1], None, OP.mult)
                    nc.vector.tensor_scalar(ao[:, (2 * t + 1) * 64:(2 * t + 2) * 64],
                                            psV[:, 64:128], rc[:, 1:2], None, OP.mult)
                transpose_in(ao, aoT, s)

            # O-proj + residual -> h
            hs = []
            for s in range(BP):
                h = res.tile([128, H], f32, tag=f"h{s}")
                for half in range(2):
                    ps = pm_mm.tile([128, 384], f32, tag="mm")
                    for kt in range(KT):
                        nc.tensor.matmul(ps, aoT[kt][:, s * 128:(s + 1) * 128],
                                         ow[kt][:, half * 384:(half + 1) * 384],
                                         start=(kt == 0), stop=False)
                    nc.tensor.matmul(ps, ones_row, obr[:, half * 384:(half + 1) * 384],
                                     start=False, stop=True)
                    nc.vector.tensor_tensor(h[:, half * 384:(half + 1) * 384], ps,
                                            xs[s][:, half * 384:(half + 1) * 384], OP.add)
                hs.append(h)

            # mid layernorm
            x2T = [actT.tile([128, 2 * 128], f16, tag=f"x2T{kt}") for kt in range(KT)]
            mids = []
            for s in range(BP):
                mu, rstd = ln_stats(hs[s])
                if prenorm:
                    xh2 = actS.tile([128, H], f16, tag="xh2")
                    nc.vector.tensor_scalar(xh2, hs[s], mu, rstd, OP.subtract, OP.mult)
                    transpose_in(xh2, x2T, s)
                    mids.append(hs[s])
                else:
                    t0 = actS.tile([128, H], f32, tag="t0")
                    nc.vector.tensor_scalar(t0, hs[s], mu, rstd, OP.subtract, OP.mult)
                    y1 = res.tile([128, H], f32, tag=f"y1_{s}")
                    nc.vector.tensor_tensor(t0, t0, grep[0], OP.mult)
                    nc.vector.tensor_tensor(y1, t0, grep[1], OP.add)
                    y116 = actS.tile([128, H], f16, tag="y116")
                    nc.vector.tensor_copy(y116, y1)
                    transpose_in(y116, x2T, s)
                    mids.append(y1)

            # FFN: FF1 (feature-major, both seqs) -> gelu -> FF2 (token-major)
            g1T = [actT.tile([128, 2 * 128], f16, tag=f"g1T{kt}") for kt in range(FKT)]
            for nt in range(FKT):
                ps = pm_mm.tile([128, 256], f32, tag="mm")
                for kt in range(KT):
                    nc.tensor.matmul(ps, f1w[kt][:, nt * 128:(nt + 1) * 128], x2T[kt],
                                     start=(kt == 0), stop=(kt == KT - 1))
                nc.scalar.activation(g1T[nt], ps, AF.Gelu_apprx_tanh,
                                     bias=f1b[:, nt:nt + 1])
            outs = []
            for s in range(BP):
                xn = res.tile([128, H], f32, tag=f"x{s}" if prenorm else f"s2_{s}")
                for half in range(2):
                    ps = pm_mm.tile([128, 384], f32, tag="mm")
                    for kt in range(FKT):
                        nc.tensor.matmul(ps, g1T[kt][:, s * 128:(s + 1) * 128],
                                         f2w[kt][:, half * 384:(half + 1) * 384],
                                         start=(kt == 0), stop=False)
                    nc.tensor.matmul(ps, ones_row, f2br[:, half * 384:(half + 1) * 384],
                                     start=False, stop=True)
                    nc.vector.tensor_tensor(xn[:, half * 384:(half + 1) * 384], ps,
                                            mids[s][:, half * 384:(half + 1) * 384], OP.add)
                outs.append(xn)

            if prenorm:
                return outs, None, None
            # post-norm LN2 -> y2 (residual); y2 f16 + transposed for next layer
            yTs2 = [actT.tile([128, 2 * 128], f16, tag=f"yT{kt}") for kt in range(KT)]
            outs2, y16s = [], []
            for s in range(BP):
                mu, rstd = ln_stats(outs[s])
                t0 = actS.tile([128, H], f32, tag="t0b")
                nc.vector.tensor_scalar(t0, outs[s], mu, rstd, OP.subtract, OP.mult)
                y2 = res.tile([128, H], f32, tag=f"y{s}")
                nc.vector.tensor_tensor(t0, t0, grep[2], OP.mult)
                nc.vector.tensor_tensor(y2, t0, grep[3], OP.add)
                y216 = actT.tile([128, H], f16, tag=f"y216_{s}")
                nc.vector.tensor_copy(y216, y2)
                transpose_in(y216, yTs2, s)
                outs2.append(y2)
                y16s.append(y216)
            return outs2, y16s, yTs2

        # ---- load embedded inputs ----
        xs = []
        for s in range(BP):
            x = res.tile([128, H], f32, tag=f"x{s}")
            nc.sync.dma_start(x, d['x0'][s])
            xs.append(x)

        # ---- contextual (pre-norm) encoder ----
        for l in range(l_ctx):
            xs, _, _ = encoder_layer('c', l, xs, None, prenorm=True)

        # ---- ctx heads ----
        ctx16 = []
        ctxT = [persist.tile([128, 2 * 128], f16, tag=f"ctxT{kt}") for kt in range(KT)]
        for s in range(BP):
            c16 = persist.tile([128, H], f16, tag=f"ctx16_{s}")
            nc.vector.tensor_copy(c16, xs[s])
            transpose_in(c16, ctxT, s)
            ctx16.append(c16)

        wbt = const.tile([128, KT], f16, tag="wbt")
        nc.sync.dma_start(wbt, d['wb'].rearrange("(k p) o -> p (k o)", p=128))
        we1t = const.tile([128, KT], f16, tag="we1t")
        nc.sync.dma_start(we1t, d['we1'].rearrange("(k p) o -> p (k o)", p=128))
        we2t = const.tile([128, KT], f16, tag="we2t")
        nc.sync.dma_start(we2t, d['we2'].rearrange("(k p) o -> p (k o)", p=128))
        clst = const.tile([128, KT, 2], f16, tag="clst")
        nc.sync.dma_start(clst, d['cls'].rearrange("(k p) o -> p k o", p=128))

        psB = pm_sm.tile([1, 256], f32, tag="hd")
        for kt in range(KT):
            nc.tensor.matmul(psB, wbt[:, kt:kt + 1], ctxT[kt],
                             start=(kt == 0), stop=(kt == KT - 1))
        begin_sb = persist.tile([1, 256], f32, tag="begin_sb")
        nc.vector.tensor_copy(begin_sb, psB)
        nc.sync.dma_start(d['o_begin'].rearrange("s t -> (s t)")[None, :], begin_sb)

        t1_sb = persist.tile([1, 256], f32, tag="t1_sb")
        t2_sb = persist.tile([1, 256], f32, tag="t2_sb")
        psT1 = pm_sm.tile([1, 256], f32, tag="hd")
        for kt in range(KT):
            nc.tensor.matmul(psT1, we1t[:, kt:kt + 1], ctxT[kt],
                             start=(kt == 0), stop=(kt == KT - 1))
        nc.vector.tensor_copy(t1_sb, psT1)
        psT2 = pm_sm.tile([1, 256], f32, tag="hd")
        for kt in range(KT):
            nc.tensor.matmul(psT2, we2t[:, kt:kt + 1], ctxT[kt],
                             start=(kt == 0), stop=(kt == KT - 1))
        nc.vector.tensor_copy(t2_sb, psT2)

        for s in range(BP):
            vals8 = sm.tile([1, 8], f32, tag="vals8")
            idx8 = sm.tile([1, 8], u32, tag="idx8")
            nc.vector.max(vals8, begin_sb[:, s * 128:(s + 1) * 128])
            nc.vector.max_index(idx8, vals8, begin_sb[:, s * 128:(s + 1) * 128])
            t1p = persist.tile([1, 160], f32, tag=f"t1p{s}")
            nc.vector.tensor_copy(t1p[:, 0:128], t1_sb[:, s * 128:(s + 1) * 128])
            lastv = t1_sb[0:1, s * 128 + 127:s * 128 + 128].to_broadcast((1, 32))
            nc.vector.tensor_copy(t1p[:, 128:160], lastv)
            t2p = persist.tile([1, 128], f32, tag=f"t2p{s}")
            nc.vector.tensor_copy(t2p, t2_sb[:, s * 128:(s + 1) * 128])
            end_sb = sm.tile([1, 32], f32, tag="end_sb")
            with tc.tile_critical():
                pv = nc.vector.value_load(idx8[0:1, 0:1], min_val=0, max_val=127)
                nc.vector.tensor_scalar(end_sb, t1p[0:1, bass.ds(pv, 32)],
                                        t2p[0:1, bass.ds(pv, 1)], None, OP.add)
            nc.sync.dma_start(d['o_end'][s][None, :], end_sb)

        for s in range(BP):
            psC = pm_sm.tile([2, 1], f32, tag="hd")
            for kt in range(KT):
                nc.tensor.matmul(psC, clst[:, kt], ctxT[kt][:, s * 128:s * 128 + 1],
                                 start=(kt == 0), stop=(kt == KT - 1))
            na = sm.tile([2, 1], f32, tag="na")
            nc.vector.tensor_copy(na, psC)
            nc.sync.dma_start(d['o_noans'][s][None, :].rearrange("a b -> b a"), na)

        # gather ctx span rows -> rowsT cols 8:16
        rowsT = [persist.tile([128, 16], f16, tag=f"rowsT{kt}") for kt in range(KT)]
        selt = []
        for i in range(BP):
            st_ = const.tile([128, SPAN], f16, tag=f"sel{i}")
            nc.sync.dma_start(st_, d['sel'][i])
            selt.append(st_)
        for i in range(BP):
            for kt in range(KT):
                psG = pm_sm.tile([128, SPAN], f32, tag="hd")
                nc.tensor.matmul(psG, ctx16[i][:, kt * 128:(kt + 1) * 128], selt[i],
                                 start=True, stop=True)
                nc.vector.tensor_copy(rowsT[kt][:, 8 + i * SPAN:8 + (i + 1) * SPAN], psG)

        # ---- BERT (post-norm) encoder ----
        ys = []
        for s in range(BP):
            y = res.tile([128, H], f32, tag=f"y{s}")
            nc.sync.dma_start(y, d['y0'][s])
            ys.append(y)
        y16s = []
        yTs = [actT.tile([128, 2 * 128], f16, tag=f"yT{kt}") for kt in range(KT)]
        for s in range(BP):
            y16 = actT.tile([128, H], f16, tag=f"y216_{s}")
            nc.vector.tensor_copy(y16, ys[s])
            transpose_in(y16, yTs, s)
            y16s.append(y16)
        for l in range(l_bert):
            ys, y16s, yTs = encoder_layer('b', l, ys, yTs, prenorm=False)

        # gather bert span rows -> rowsT cols 0:8
        for i in range(BP):
            for kt in range(KT):
                psG = pm_sm.tile([128, SPAN], f32, tag="hd")
                nc.tensor.matmul(psG, y16s[i][:, kt * 128:(kt + 1) * 128], selt[i],
                                 start=True, stop=True)
                nc.vector.tensor_copy(rowsT[kt][:, i * SPAN:(i + 1) * SPAN], psG)

        # ---- DMLM projection + softmax (16 rows x V) ----
        exps = persist.tile([16, V], f16, tag="exps")
        sumc = persist.tile([16, NVT], f32, tag="sumc")
        NCH = 2  # column tiles per weight DMA chunk
        for c0 in range(0, NVT, NCH):
            cw = min(NCH * VTILE, V - c0 * VTILE)
            pwc = [wp.tile([128, NCH * VTILE], f16, tag=f"pwc{kt}") for kt in range(KT)]
            for kt in range(KT):
                nc.sync.dma_start(pwc[kt][:, 0:cw],
                                  d['pw'][kt * 128:(kt + 1) * 128,
                                          c0 * VTILE:c0 * VTILE + cw])
            pbc = wp.tile([1, NCH * VTILE], f16, tag="pbc")
            nc.sync.dma_start(pbc[:, 0:cw], d['pb'][:, c0 * VTILE:c0 * VTILE + cw])
            for nt in range(c0, min(c0 + NCH, NVT)):
                n = min(VTILE, V - nt * VTILE)
                off = (nt - c0) * VTILE
                ps = pm_mm.tile([16, VTILE], f32, tag="mm")
                for kt in range(KT):
                    nc.tensor.matmul(ps[:, 0:n], rowsT[kt], pwc[kt][:, off:off + n],
                                     start=(kt == 0), stop=False)
                nc.tensor.matmul(ps[:, 0:n], ones_16, pbc[:, off:off + n],
                                 start=False, stop=True)
                nc.scalar.activation(exps[:, nt * VTILE:nt * VTILE + n], ps[:, 0:n],
                                     AF.Exp, accum_out=sumc[:, nt:nt + 1])
        tsum = sm.tile([16, 1], f32, tag="tsum")
        nc.vector.reduce_sum(tsum, sumc, axis=mybir.AxisListType.X)
        rsc = sm.tile([16, 1], f32, tag="rsc")
        nc.vector.reciprocal(rsc, tsum)
        nc.vector.tensor_scalar(exps, exps, rsc, None, OP.mult)
        nc.sync.dma_start(d['o_bert'], exps[0:8])
        nc.sync.dma_start(d['o_ctx'], exps[8:16])

    nc.compile()
    return nc


def _host_prep(inputs):
    """Fold LN params into weights (fp64), cast fp16, build per-core input maps."""
    p = inputs['params']
    ids_m = np.asarray(inputs['input_ids_msk'])
    ids = np.asarray(inputs['input_ids'])
    tts = np.asarray(inputs['token_type_ids'])
    beg = np.asarray(inputs['begin_label'])

    f64 = np.float64
    c = {k: np.asarray(v, f64) for k, v in p['ctx'].items()}
    b = {k: np.asarray(v, f64) for k, v in p['bert'].items()}

    # embeddings + bert embedding layernorm (host)
    pos = np.arange(S)
    x0 = (np.asarray(p['tok_emb'], f64)[ids_m]
          + np.asarray(p['pos_emb'], f64)[pos][None]
          + np.asarray(p['seg_emb'], f64)[1][None, None])
    y0p = (b['tok_emb'][ids] + b['pos_emb'][pos][None] + b['type_emb'][tts])
    mu = y0p.mean(-1, keepdims=True)
    var = ((y0p - mu) ** 2).mean(-1, keepdims=True)
    y0 = (y0p - mu) / np.sqrt(var + 1e-12) * b['emb_ln_g'] + b['emb_ln_b']
    x0 = x0.astype(np.float32)
    y0 = y0.astype(np.float32)

    def pack(pfx, qkv_w, qkv_b, o_w, o_b, f1w, f1b, f2w, f2b, ll):
        qk_w = qkv_w[:, :, :2 * H]
        v_w = qkv_w[:, :, 2 * H:]
        qk_b = qkv_b[:, :2 * H]
        v_b = qkv_b[:, 2 * H:]
        ob = o_b + np.einsum('lk,lkn->ln', v_b, o_w)
        return {
            pfx + 'qkw': qk_w.astype(np.float16),
            pfx + 'qkb': qk_b.reshape(ll, 12, 128).transpose(0, 2, 1).astype(np.float32).copy(),
            pfx + 'vw': v_w.astype(np.float16),
            pfx + 'ow': o_w.astype(np.float16),
            pfx + 'ob': ob.astype(np.float16),
            pfx + 'f1w': f1w.astype(np.float16),
            pfx + 'f1b': f1b.reshape(ll, FKT, 128).transpose(0, 2, 1).astype(np.float32).copy(),
            pfx + 'f2w': f2w.astype(np.float16),
            pfx + 'f2b': f2b.astype(np.float16),
        }

    lc, lb = L_CTX, L_BERT
    qkv_w = c['ln1_g'][:lc, :, None] * c['qkv_w'][:lc]
    qkv_b = c['qkv_b'][:lc] + np.einsum('lk,lkn->ln', c['ln1_b'][:lc], c['qkv_w'][:lc])
    f1w = c['ln2_g'][:lc, :, None] * c['ff1_w'][:lc]
    f1b = c['ff1_b'][:lc] + np.einsum('lk,lkn->ln', c['ln2_b'][:lc], c['ff1_w'][:lc])
    wts = pack('c', qkv_w, qkv_b, c['o_w'][:lc], c['o_b'][:lc],
               f1w, f1b, c['ff2_w'][:lc], c['ff2_b'][:lc], lc)
    wts.update(pack('b', b['qkv_w'][:lb], b['qkv_b'][:lb], b['o_w'][:lb], b['o_b'][:lb],
                    b['ff1_w'][:lb], b['ff1_b'][:lb], b['ff2_w'][:lb], b['ff2_b'][:lb], lb))
    wts['bln'] = np.stack([b['ln1_g'][:lb], b['ln1_b'][:lb],
                           b['ln2_g'][:lb], b['ln2_b'][:lb]], 1).astype(np.float32)
    wts['wb'] = np.asarray(p['wb_w']).astype(np.float16)
    we = np.asarray(p['we_w']).astype(np.float16)
    wts['we1'] = we[:H]
    wts['we2'] = we[H:]
    wts['cls'] = np.asarray(p['cls_w']).astype(np.float16)
    wts['pw'] = np.asarray(p['proj_w']).astype(np.float16)
    wts['pb'] = np.asarray(p['proj_b']).astype(np.float16).reshape(1, V)

    in_maps = []
    for cidx in range(NCORES):
        sl = slice(cidx * BP, (cidx + 1) * BP)
        sel = np.zeros((BP, S, SPAN), np.float16)
        bsl = beg[sl]
        for i in range(BP):
            for j in range(SPAN):
                sel[i, bsl[i] + j, j] = 1.0
        m = dict(wts)
        m['x0'] = np.ascontiguousarray(x0[sl])
        m['y0'] = np.ascontiguousarray(y0[sl])
        m['sel'] = sel
        in_maps.append(m)
    return in_maps


def kernel(**inputs):
    from concourse import bass_utils
    key = (L_CTX, L_BERT)
    if key not in _BUILT:
        _BUILT[key] = _build(*key)
    nc = _BUILT[key]
    in_maps = _host_prep(inputs)
    res = bass_utils.run_bass_kernel_spmd(nc, in_maps, core_ids=list(range(NCORES)))
    p = inputs['params']
    wb_b = np.float32(np.asarray(p['wb_b'])[0])
    we_b = np.float32(np.asarray(p['we_b'])[0])
    cls_b = np.asarray(p['cls_b'], np.float32)
    begin = np.concatenate([r['o_begin'] for r in res.results], 0) + wb_b
    end = np.concatenate([r['o_end'] for r in res.results], 0) + we_b
    noans = np.concatenate([r['o_noans'] for r in res.results], 0) + cls_b[None, :]
    bert_gt = np.concatenate([r['o_bert'] for r in res.results], 0).astype(np.float32)
    ctx_pred = np.concatenate([r['o_ctx'] for r in res.results], 0).astype(np.float32)
    return begin.astype(np.float32), end.astype(np.float32), noans.astype(np.float32), \
        bert_gt, ctx_pred
